# revision 4
# baseline (speedup 1.0000x reference)
"""DeepSeek-MoE SwiGLU expert layer on 8 TRN2 NeuronCores (expert parallelism).

Strategy (hardcoded for T=4096, D=1024, DFF=1408, E=8, K=2, 8 cores):
  - Expert parallelism: core e holds expert e's (Wg, Wu, Wd).
  - Dispatch happens at input-sharding time on the host: for each expert,
    gather the tokens routed to it (deduped via the combine matrix), pad to
    capacity C, and ship X^T pre-tiled so every device DMA is a contiguous
    [128, bytes] stream and every matmul operand is natural-layout.
  - All matmul operands are bf16: the PE streams 16-bit moving data at ~2
    cols/cycle (~2x fp32r throughput measured on HW); fp32 PSUM accumulation
    keeps absmax rel err ~6e-3 (gate is 2e-2).
  - Each PSUM accumulation group is one long uninterrupted matmul chain
    (interleaving groups per-matmul flushes the PE pipeline and costs ~70%).
  - Weights are DMA'd once into SBUF and stay resident.  x streams in via
    the gpsimd (SWDGE) queue, which has no other work and therefore prefetches
    a full iteration ahead in the bench loop; y streams out via the SP HWDGE
    queue; wu preloads via the Act HWDGE queue in parallel with wg on SP.
  - Per core:  HT = silu(Wg^T @ XT) * (Wu^T @ XT)   [DFF, C]  (h stored bf16)
               YT = Wd^T @ HT                        [D, C]   (shipped bf16)
  - Combine on host: out[idx_e] += (YT[:, :cnt]).T * combine_weight.
"""

import numpy as np
import ml_dtypes
from contextlib import ExitStack

import concourse.bass as bass
import concourse.tile as tile
from concourse import bacc, mybir
from concourse import bass_utils

T, D, DFF, E = 4096, 1024, 1408, 8
N_CORES = 8
P = 128
KD = D // P    # 8 k-tiles over D
KF = DFF // P  # 11 k-tiles over DFF
CT = 512       # max matmul moving-operand width (one PSUM bank of fp32)
BF_NP = ml_dtypes.bfloat16

_cache = {}


def _c_tiles(C):
    tiles = []
    off = 0
    while off < C:
        w = min(CT, C - off)
        tiles.append((off, w))
        off += w
    return tiles


def _declare(nc, C):
    bf = mybir.dt.bfloat16
    # host-pre-tiled layouts; every DMA below is contiguous per partition
    xt = nc.dram_tensor("xt", [P, KD, C], bf, kind="ExternalInput").ap()
    wg = nc.dram_tensor("wg", [P, KF, KD, P], bf, kind="ExternalInput").ap()
    wu = nc.dram_tensor("wu", [P, KF, KD, P], bf, kind="ExternalInput").ap()
    wd = nc.dram_tensor("wd", [P, KD, KF, P], bf, kind="ExternalInput").ap()
    yt = nc.dram_tensor("yt", [KD, P, C], bf, kind="ExternalOutput").ap()
    return (xt, wg, wu, wd, yt)


def _pools(tc, ctx):
    wp = ctx.enter_context(tc.tile_pool(name="w_p", bufs=1))
    xp = ctx.enter_context(tc.tile_pool(name="xt_p", bufs=2))
    hp = ctx.enter_context(tc.tile_pool(name="ht_p", bufs=1))
    pp = ctx.enter_context(tc.tile_pool(name="ps_p", bufs=2, space="PSUM"))
    sp = ctx.enter_context(tc.tile_pool(name="sg_p", bufs=4))
    op = ctx.enter_context(tc.tile_pool(name="y_p", bufs=4))
    return (wp, xp, hp, pp, sp, op)


def _emit_weights(nc, pools, aps):
    """Preload all expert weights into SBUF (resident).  wg on the SP queue,
    wu on the Act queue (parallel), wd on SP after wg (needed only by
    stage 2).  Sliced so the first matmul group can start early."""
    bf = mybir.dt.bfloat16
    wp = pools[0]
    _, wg, wu, wd, _ = aps
    wg_sb = wp.tile([P, KF, KD, P], bf, tag="wg", name="wg_sb")
    wu_sb = wp.tile([P, KF, KD, P], bf, tag="wu", name="wu_sb")
    wd_sb = wp.tile([P, KD, KF, P], bf, tag="wd", name="wd_sb")
    for f in range(KF):
        nc.sync.dma_start(out=wg_sb[:, f], in_=wg[:, f])
        nc.scalar.dma_start(out=wu_sb[:, f], in_=wu[:, f])
    for do in range(KD):
        nc.sync.dma_start(out=wd_sb[:, do], in_=wd[:, do])
    return wg_sb, wu_sb, wd_sb


def _emit_compute(nc, pools, aps, w_sb, C):
    bf = mybir.dt.bfloat16
    f32 = mybir.dt.float32
    ctiles = _c_tiles(C)
    wp, xp, hp, pp, sp, op = pools
    xt, wg, wu, wd, yt = aps
    wg_sb, wu_sb, wd_sb = w_sb
    Silu = mybir.ActivationFunctionType.Silu
    Copy = mybir.ActivationFunctionType.Copy

    # x prefetch on the gpsimd SWDGE queue: no other work there, so in the
    # bench loop it runs a full iteration ahead of the compute.
    x_sb = {}
    for i, (c0, cw) in enumerate(ctiles):
        x_sb[i] = xp.tile([P, KD, cw], bf, tag=f"x{i}", name=f"x_sb{i}")
        nc.gpsimd.dma_start(out=x_sb[i][:], in_=xt[:, :, c0:c0 + cw])

    h_sb = {}
    for i, (c0, cw) in enumerate(ctiles):
        h_sb[i] = hp.tile([P, KF, cw], bf, tag=f"h{i}", name=f"h_sb{i}")

    # stage 1: HT[f, c] = silu(Wg^T XT) * (Wu^T XT), transposed space
    for i, (c0, cw) in enumerate(ctiles):
        for f in range(KF):
            ps_g = pp.tile([P, cw], f32, tag="psg", name="ps_g")
            ps_u = pp.tile([P, cw], f32, tag="psu", name="ps_u")
            for k in range(KD):
                nc.tensor.matmul(ps_g[:], lhsT=wg_sb[:, f, k],
                                 rhs=x_sb[i][:, k],
                                 start=(k == 0), stop=(k == KD - 1))
            for k in range(KD):
                nc.tensor.matmul(ps_u[:], lhsT=wu_sb[:, f, k],
                                 rhs=x_sb[i][:, k],
                                 start=(k == 0), stop=(k == KD - 1))
            sg = sp.tile([P, cw], f32, name="sg")
            nc.scalar.activation(sg[:], ps_g[:], Silu)
            nc.vector.tensor_mul(h_sb[i][:, f], sg[:], ps_u[:])

    # stage 2: YT[dout, c] = Wd^T @ HT
    for i, (c0, cw) in enumerate(ctiles):
        for do in range(KD):
            ps_y = pp.tile([P, cw], f32, tag="psy", name="ps_y")
            for k in range(KF):
                nc.tensor.matmul(ps_y[:], lhsT=wd_sb[:, do, k],
                                 rhs=h_sb[i][:, k],
                                 start=(k == 0), stop=(k == KF - 1))
            y_sb = op.tile([P, cw], bf, name="y_sb")
            nc.scalar.activation(y_sb[:], ps_y[:], Copy)
            nc.sync.dma_start(out=yt[do, :, c0:c0 + cw], in_=y_sb[:])


def _build(C):
    key = ("plain", C)
    if key in _cache:
        return _cache[key]
    nc = bacc.Bacc("TRN2", target_bir_lowering=False, debug=False,
                   num_devices=N_CORES)
    aps = _declare(nc, C)
    with tile.TileContext(nc) as tc, ExitStack() as ctx:
        pools = _pools(tc, ctx)
        w_sb = _emit_weights(nc, pools, aps)
        _emit_compute(nc, pools, aps, w_sb, C)
    nc.compile()
    _cache[key] = nc
    return nc


def _build_loop(C):
    """Benchmark variant: weights resident, body repeated niter times."""
    key = ("loop", C)
    if key in _cache:
        return _cache[key]
    nc = bacc.Bacc("TRN2", target_bir_lowering=False, debug=False,
                   num_devices=N_CORES)
    aps = _declare(nc, C)
    n_ap = nc.dram_tensor("niter", [1, 1], mybir.dt.uint32,
                          kind="ExternalInput").ap()
    with tile.TileContext(nc) as tc, ExitStack() as ctx:
        cpool = ctx.enter_context(tc.tile_pool(name="c_p", bufs=1))
        pools = _pools(tc, ctx)
        n_sb = cpool.tile([1, 1], mybir.dt.uint32)
        nc.sync.dma_start(out=n_sb[:], in_=n_ap[:])
        w_sb = _emit_weights(nc, pools, aps)
        with tc.tile_critical():
            tmp = nc.alloc_registers("niter_regs")
            nc.regs_load(tmp, n_sb[0:1, 0:1])
            n_val = nc.snap(tmp, donate=True, min_val=0, max_val=1 << 20)
        with tc.For_i(0, n_val, 1, hint_engines=(mybir.EngineType.PE,)):
            _emit_compute(nc, pools, aps, w_sb, C)
    nc.compile()
    _cache[key] = nc
    return nc


def _dispatch(x, topk_weights, topk_indices, num_experts):
    """Host-side routing: combine matrix + per-expert token index lists."""
    T_, _ = x.shape
    E_ = int(num_experts)
    ti = np.asarray(topk_indices).astype(np.int64)
    tw = np.asarray(topk_weights).astype(np.float32)
    combine = np.zeros((T_, E_), np.float32)
    np.add.at(combine, (np.arange(T_)[:, None], ti), tw)
    idxs = [np.nonzero(combine[:, e])[0] for e in range(E_)]
    return combine, idxs


def _capacity(idxs):
    maxc = max((len(i) for i in idxs), default=0)
    return max(2 * P, ((maxc + 31) // 32) * 32)


def _tile_w1(W):
    # [D, DFF] -> [P, KF, KD, P]: w[p, f, k, m] = W[k*P+p, f*P+m]
    return np.ascontiguousarray(
        W.reshape(KD, P, KF, P).transpose(1, 2, 0, 3)).astype(BF_NP)


def _tile_w2(W):
    # [DFF, D] -> [P, KD, KF, P]: w[p, do, k, m] = W[k*P+p, do*P+m]
    return np.ascontiguousarray(
        W.reshape(KF, P, KD, P).transpose(1, 2, 0, 3)).astype(BF_NP)


def _in_maps(x, Wg, Wu, Wd, idxs, C):
    maps = []
    for e in range(len(idxs)):
        n = len(idxs[e])
        xe = np.zeros((C, KD, P), np.float32)
        xe[:n] = x[idxs[e]].reshape(n, KD, P)
        # xt[p, k, c] = x[idx[c], k*P+p]
        xt_e = np.ascontiguousarray(xe.transpose(2, 1, 0)).astype(BF_NP)
        maps.append({
            "xt": xt_e,
            "wg": _tile_w1(np.asarray(Wg[e], np.float32)),
            "wu": _tile_w1(np.asarray(Wu[e], np.float32)),
            "wd": _tile_w2(np.asarray(Wd[e], np.float32)),
        })
    return maps


def kernel(x, Wg, Wu, Wd, topk_weights, topk_indices, num_experts):
    x = np.asarray(x, np.float32)
    Wg = np.asarray(Wg, np.float32)
    Wu = np.asarray(Wu, np.float32)
    Wd = np.asarray(Wd, np.float32)
    T_, D_ = x.shape

    combine, idxs = _dispatch(x, topk_weights, topk_indices, num_experts)
    C = _capacity(idxs)

    nc = _build(C)
    res = bass_utils.run_bass_kernel_spmd(nc, _in_maps(x, Wg, Wu, Wd, idxs, C),
                                          list(range(N_CORES)))

    out = np.zeros((T_, D_), np.float32)
    for e in range(len(idxs)):
        n = len(idxs[e])
        if n:
            ye = res.results[e]["yt"].reshape(D_, C)[:, :n].astype(np.float32).T
            out[idxs[e]] += ye * combine[idxs[e], e][:, None]
    return out
